# revision 34
# baseline (speedup 1.0000x reference)
"""Trainium2 Bass kernel for nn_ConvLayer: 3x3 conv (stride 1, pad 1) + per-channel offset.

Problem: x[32,64,56,56] (*) w[128,64,3,3] + offset[128,1,1] -> out[32,128,56,56], fp32.

Strategy (8 NeuronCores, data-parallel over batch, 4 images/core):
  - Conv as 9 shifted matmuls (one per 3x3 tap) accumulated in PSUM.
  - CIN=64 -> each tap is a contract-64 matmul = half the 128x128 PE array.
    Two images are processed CONCURRENTLY via 64x128 row tiling: image A's
    channels live in SBUF partitions 0-63 (PE tile (0,0)), image B's in
    partitions 64-127 (PE tile (64,0)). Each accumulates into its own PSUM
    bank, reaching full PE-array packing with no data duplication.
  - All HBM traffic is bf16 (half of fp32): x and weights are pre-cast on
    the host, the output is stored bf16 and widened to fp32 on the host.
    bf16 matmuls run at the same 1 cycle/row rate as fp32r but get FWL
    (fast weight load) and halve every DMA.
  - Host pre-pads x to a 58x58 grid (zeros on borders) so every tap is a
    contiguous shifted window; host pre-transposes the weight to [cin,tap,k]
    (lhsT layout) and duplicates it into both partition halves.
  - Output columns are produced on the padded 58-wide grid; the PSUM->SBUF
    eviction (all on VectorE) compacts back to the dense 56-wide grid and
    fuses the per-channel offset add, so the store DMA is fully contiguous.
    ScalarE runs no compute at all: that keeps the framework from prepending
    an ACT_TABLE_LOAD to the ACT HWDGE ring, so the weight DMA starts there
    immediately while x streams on the SP ring in parallel.
  - TensorE warmup matmuls bridge the input-DMA head so the HAM clock gate
    (1.2 -> 2.4 GHz after ~3.4us of sustained PE activity) opens early.
  - DMA triggers are expensive (~0.6-1us of issuing-engine time each), so
    transfers are few and large: 4 input slices for the first pair, 1 for
    the second, 3 output slices per image.
"""

import numpy as np
from contextlib import ExitStack

import ml_dtypes

import concourse.bass as bass
import concourse.tile as tile
from concourse import bacc, mybir
from concourse.bass_utils import run_bass_kernel_spmd

# Problem constants (hardcoded per contract).
B, CIN, HW, K = 32, 64, 56, 128
NCORES = 8
BPC = B // NCORES          # images per core
HP = HW + 2                # padded row width: 58
NPAD = HP * HP + 4         # padded image + slack for tap reads: 3368
NOUT = HW * HW             # 3136
ROWS_PER_CHUNK = 8
CHUNK = ROWS_PER_CHUNK * HP     # 464 <= 512 (one PSUM bank, fp32)
DCHUNK = ROWS_PER_CHUNK * HW    # 448 dense output cols per chunk
NCHUNKS = HW // ROWS_PER_CHUNK  # 7
TAPS = 9
F32 = mybir.dt.float32
BF16 = mybir.dt.bfloat16
NPBF16 = ml_dtypes.bfloat16

# Warmup matmuls: keep TensorE busy from engine-start until the first real
# matmul's input lands, so the HAM clock gate (1.2 -> 2.4 GHz after ~3.4us of
# sustained PE activity) opens as early as possible. The PE queue is FIFO, so
# the count must undershoot the data-arrival time or warmups delay real work.
WARMUP_N = 256
WARMUPS = 12

_NC_CACHE = None


def _conv_kernel(ctx: ExitStack, tc: "tile.TileContext", out_ap, xp_ap, w2_ap, off_ap):
    nc = tc.nc
    singles = ctx.enter_context(tc.tile_pool(name="singles", bufs=1))
    xpool = ctx.enter_context(tc.tile_pool(name="xpool", bufs=2))
    opool = ctx.enter_context(tc.tile_pool(name="opool", bufs=2))
    psum = ctx.enter_context(tc.tile_pool(name="psum", bufs=8, space="PSUM"))

    # Chunk groups: first group is a single chunk so its input slice is small
    # and the first matmul starts as early as possible; later groups pair
    # chunks to amortize weight loads. 4 PSUM banks max per group, 8 total
    # with double buffering.
    groups = [(0,), (1, 2), (3, 4), (5,), (6,)]
    # x-load slices (pair 0 only; pair 1 is one bulk transfer). Slice 0 is
    # exactly chunk 0's window so the first matmul gates on a minimal load.
    xbounds = [0, 584, 1512, 2440, NPAD]
    # Output store slices (dense cols), issued after chunk groups 2/3/4; the
    # small last slice keeps the post-compute drain short.
    obounds = [0, 4 * DCHUNK, 6 * DCHUNK, NOUT]
    ogroup = [2, 3, 4]

    # Weights ride the ACT HWDGE ring and x the SP ring, so the two
    # first-matmul inputs stream concurrently from the earliest possible
    # moment. This only works because no ScalarE compute op exists in this
    # kernel (evictions are all on DVE): an activation op would make the
    # framework prepend a ~1.3us ACT_TABLE_LOAD that blocks the ACT ring.
    w_sb = singles.tile([128, TAPS, K], BF16)
    nc.scalar.dma_start(w_sb[:], w2_ap[:])
    off_sb = singles.tile([128, 1], F32)
    nc.scalar.dma_start(off_sb[:], off_ap[:])

    # PE warmup (see WARMUPS above). The memset rides GpSimd, whose queue is
    # free right after the start barrier (Vector would gate the first warmup
    # ~0.5us later). A few short trailing warmups keep the bridge to the
    # first real matmul fine-grained.
    scratch = singles.tile([128, WARMUP_N], BF16)
    nc.gpsimd.memset(scratch[:], 0.0)
    ps_warm = psum.tile([128, WARMUP_N], F32, tag="ps", name="ps_warm")
    for _ in range(WARMUPS):
        nc.tensor.matmul(
            ps_warm[:], lhsT=scratch[0:64, 0:128], rhs=scratch[0:64, :],
            start=True, stop=True,
        )
    for _ in range(10):
        nc.tensor.matmul(
            ps_warm[:, 0:64], lhsT=scratch[0:64, 0:128], rhs=scratch[0:64, 0:64],
            start=True, stop=True,
        )

    for pair in range(BPC // 2):
        b0 = 2 * pair
        # Both images of the pair side by side: [2, CIN, NPAD] -> [128, NPAD].
        x_t = xpool.tile([128, NPAD], BF16, tag="x")
        xsrc = xp_ap[b0 : b0 + 2].rearrange("b c n -> (b c) n")
        if pair == 0:
            for s in range(len(xbounds) - 1):
                nc.sync.dma_start(
                    x_t[:, xbounds[s] : xbounds[s + 1]],
                    xsrc[:, xbounds[s] : xbounds[s + 1]],
                )
        else:
            # Pair 1's bulk load rides the ACT ring (idle after the weight
            # load) so the SP ring stays free for pair-0 slices and the
            # first output stores.
            nc.scalar.dma_start(x_t[:], xsrc[:])
        o_sb = [
            opool.tile([128, NOUT], BF16, tag="oA", name=f"oA_{pair}"),
            opool.tile([128, NOUT], BF16, tag="oB", name=f"oB_{pair}"),
        ]

        for g, grp in enumerate(groups):
            ps = {}
            for half in (0, 1):
                for c in grp:
                    ps[(half, c)] = psum.tile(
                        [128, CHUNK], F32, tag="ps", name=f"ps_{pair}_{half}_{c}"
                    )
            for t in range(TAPS):
                kh, kw = divmod(t, 3)
                o = kh * HP + kw
                st, sp = (t == 0), (t == TAPS - 1)
                for half in (0, 1):
                    lo, hi = 64 * half, 64 * half + 64
                    for c in grp:
                        nc.tensor.matmul(
                            ps[(half, c)][:],
                            lhsT=w_sb[lo:hi, t, :],
                            rhs=x_t[lo:hi, o + CHUNK * c : o + CHUNK * c + CHUNK],
                            start=st,
                            stop=sp,
                        )
            # Evict: compact 58-wide padded rows to 56-wide dense rows, add
            # the per-channel offset, and cast to bf16. Both images on DVE —
            # keeping ScalarE free of compute keeps the ACT DMA ring
            # unblocked (see the weight load above).
            # Image B first: its store rides the busier SP ring, so giving
            # its eviction the earlier DVE slot lets that store trigger
            # sooner (matters for the final chunk's drain).
            for c in grp:
                pb = ps[(1, c)].rearrange("p (r x) -> p r x", x=HP)[:, :, 0:HW]
                ob = o_sb[1][:, c * DCHUNK : (c + 1) * DCHUNK].rearrange(
                    "p (r x) -> p r x", x=HW
                )
                nc.vector.tensor_scalar_add(ob, pb, off_sb)
                pa = ps[(0, c)].rearrange("p (r x) -> p r x", x=HP)[:, :, 0:HW]
                oa = o_sb[0][:, c * DCHUNK : (c + 1) * DCHUNK].rearrange(
                    "p (r x) -> p r x", x=HW
                )
                nc.vector.tensor_scalar_add(oa, pa, off_sb)
            # Stream completed output slices out. Image A rides the ACT ring
            # (idle apart from evictions by the time stores begin), image B
            # the SP ring behind the input loads.
            if g in ogroup:
                si = ogroup.index(g)
                nc.scalar.dma_start(
                    out_ap[b0][:, obounds[si] : obounds[si + 1]],
                    o_sb[0][:, obounds[si] : obounds[si + 1]],
                )
                nc.sync.dma_start(
                    out_ap[b0 + 1][:, obounds[si] : obounds[si + 1]],
                    o_sb[1][:, obounds[si] : obounds[si + 1]],
                )


def _build_nc():
    global _NC_CACHE
    if _NC_CACHE is not None:
        return _NC_CACHE
    nc = bacc.Bacc(
        "TRN2", target_bir_lowering=False, debug=False, num_devices=NCORES
    )
    xp_ap = nc.dram_tensor("xp", [BPC, CIN, NPAD], BF16, kind="ExternalInput").ap()
    w2_ap = nc.dram_tensor("w2", [128, TAPS, K], BF16, kind="ExternalInput").ap()
    off_ap = nc.dram_tensor("off", [K, 1], F32, kind="ExternalInput").ap()
    out_ap = nc.dram_tensor("out", [BPC, K, NOUT], BF16, kind="ExternalOutput").ap()
    with tile.TileContext(nc) as tc:
        with ExitStack() as ctx:
            _conv_kernel(ctx, tc, out_ap, xp_ap, w2_ap, off_ap)
    nc.compile()
    _NC_CACHE = nc
    return nc


def _prep_inputs(x, weight, offset):
    """Host-side layout prep: pad x, transpose+duplicate weights, cast bf16."""
    x = np.asarray(x, dtype=np.float32)
    weight = np.asarray(weight, dtype=np.float32)
    offset = np.asarray(offset, dtype=np.float32)

    xph = np.zeros((B, CIN, NPAD), dtype=NPBF16)
    xph[:, :, : HP * HP].reshape(B, CIN, HP, HP)[:, :, 1 : 1 + HW, 1 : 1 + HW] = (
        x.astype(NPBF16)
    )

    wt = np.ascontiguousarray(weight.transpose(1, 2, 3, 0)).reshape(CIN, TAPS, K)
    w2 = np.concatenate([wt, wt], axis=0).astype(NPBF16)  # [128, 9, 128]
    off = np.ascontiguousarray(offset.reshape(K, 1))
    return xph, w2, off


def kernel(x, weight, offset):
    nc = _build_nc()
    xph, w2, off = _prep_inputs(x, weight, offset)
    in_maps = [
        {"xp": xph[i * BPC : (i + 1) * BPC], "w2": w2, "off": off}
        for i in range(NCORES)
    ]
    res = run_bass_kernel_spmd(nc, in_maps, list(range(NCORES))).results
    out = np.concatenate(
        [
            np.asarray(res[i]["out"]).reshape(BPC, K, HW, HW)
            for i in range(NCORES)
        ],
        axis=0,
    )
    return out.astype(np.float32)


# revision 35
# speedup vs baseline: 1.0017x; 1.0017x over previous
"""Trainium2 Bass kernel for nn_ConvLayer: 3x3 conv (stride 1, pad 1) + per-channel offset.

Problem: x[32,64,56,56] (*) w[128,64,3,3] + offset[128,1,1] -> out[32,128,56,56], fp32.

Strategy (8 NeuronCores, data-parallel over batch, 4 images/core):
  - Conv as 9 shifted matmuls (one per 3x3 tap) accumulated in PSUM.
  - CIN=64 -> each tap is a contract-64 matmul = half the 128x128 PE array.
    Two images are processed CONCURRENTLY via 64x128 row tiling: image A's
    channels live in SBUF partitions 0-63 (PE tile (0,0)), image B's in
    partitions 64-127 (PE tile (64,0)). Each accumulates into its own PSUM
    bank, reaching full PE-array packing with no data duplication.
  - All HBM traffic is bf16 (half of fp32): x and weights are pre-cast on
    the host, the output is stored bf16 and widened to fp32 on the host.
    bf16 matmuls run at the same 1 cycle/row rate as fp32r but get FWL
    (fast weight load) and halve every DMA.
  - Host pre-pads x to a 58x58 grid (zeros on borders) so every tap is a
    contiguous shifted window; host pre-transposes the weight to [cin,tap,k]
    (lhsT layout) and duplicates it into both partition halves.
  - Output columns are produced on the padded 58-wide grid; the PSUM->SBUF
    eviction (all on VectorE) compacts back to the dense 56-wide grid and
    fuses the per-channel offset add, so the store DMA is fully contiguous.
    ScalarE runs no compute at all: that keeps the framework from prepending
    an ACT_TABLE_LOAD to the ACT HWDGE ring, so the weight DMA starts there
    immediately while x streams on the SP ring in parallel.
  - TensorE warmup matmuls bridge the input-DMA head so the HAM clock gate
    (1.2 -> 2.4 GHz after ~3.4us of sustained PE activity) opens early.
  - DMA triggers are expensive (~0.6-1us of issuing-engine time each), so
    transfers are few and large: 4 input slices for the first pair, 1 for
    the second, 3 output slices per image.
"""

import numpy as np
from contextlib import ExitStack

import ml_dtypes

import concourse.bass as bass
import concourse.tile as tile
from concourse import bacc, mybir
from concourse.bass_utils import run_bass_kernel_spmd

# Problem constants (hardcoded per contract).
B, CIN, HW, K = 32, 64, 56, 128
NCORES = 8
BPC = B // NCORES          # images per core
HP = HW + 2                # padded row width: 58
NPAD = HP * HP + 4         # padded image + slack for tap reads: 3368
NOUT = HW * HW             # 3136
ROWS_PER_CHUNK = 8
CHUNK = ROWS_PER_CHUNK * HP     # 464 <= 512 (one PSUM bank, fp32)
DCHUNK = ROWS_PER_CHUNK * HW    # 448 dense output cols per chunk
NCHUNKS = HW // ROWS_PER_CHUNK  # 7
TAPS = 9
F32 = mybir.dt.float32
BF16 = mybir.dt.bfloat16
NPBF16 = ml_dtypes.bfloat16

# Warmup matmuls: keep TensorE busy from engine-start until the first real
# matmul's input lands, so the HAM clock gate (1.2 -> 2.4 GHz after ~3.4us of
# sustained PE activity) opens as early as possible. The PE queue is FIFO, so
# the count must undershoot the data-arrival time or warmups delay real work.
WARMUP_N = 256
WARMUPS = 12

_NC_CACHE = None


def _conv_kernel(ctx: ExitStack, tc: "tile.TileContext", out_ap, xp_ap, w2_ap, off_ap):
    nc = tc.nc
    singles = ctx.enter_context(tc.tile_pool(name="singles", bufs=1))
    xpool = ctx.enter_context(tc.tile_pool(name="xpool", bufs=2))
    opool = ctx.enter_context(tc.tile_pool(name="opool", bufs=2))
    psum = ctx.enter_context(tc.tile_pool(name="psum", bufs=8, space="PSUM"))

    # Chunk groups: first group is a single chunk so its input slice is small
    # and the first matmul starts as early as possible; later groups pair
    # chunks to amortize weight loads. 4 PSUM banks max per group, 8 total
    # with double buffering.
    groups = [(0,), (1, 2), (3, 4), (5,), (6,)]
    # x-load slices (pair 0 only; pair 1 is one bulk transfer). Slice 0 is
    # exactly chunk 0's window so the first matmul gates on a minimal load.
    xbounds = [0, 584, 1512, 2440, NPAD]
    # Output store slices (dense cols), issued after chunk groups 2/3/4; the
    # small last slice keeps the post-compute drain short.
    obounds = [0, 4 * DCHUNK, 6 * DCHUNK, NOUT]
    ogroup = [2, 3, 4]

    # Weights ride the ACT HWDGE ring and x the SP ring, so the two
    # first-matmul inputs stream concurrently from the earliest possible
    # moment. This only works because no ScalarE compute op exists in this
    # kernel (evictions are all on DVE): an activation op would make the
    # framework prepend a ~1.3us ACT_TABLE_LOAD that blocks the ACT ring.
    w_sb = singles.tile([128, TAPS, K], BF16)
    nc.scalar.dma_start(w_sb[:], w2_ap[:])
    off_sb = singles.tile([128, 1], F32)
    nc.scalar.dma_start(off_sb[:], off_ap[:])

    # PE warmup (see WARMUPS above). The memset rides GpSimd, whose queue is
    # free right after the start barrier (Vector would gate the first warmup
    # ~0.5us later). A few short trailing warmups keep the bridge to the
    # first real matmul fine-grained.
    scratch = singles.tile([128, WARMUP_N], BF16)
    nc.gpsimd.memset(scratch[:], 0.0)
    ps_warm = psum.tile([128, WARMUP_N], F32, tag="ps", name="ps_warm")
    for _ in range(WARMUPS):
        nc.tensor.matmul(
            ps_warm[:], lhsT=scratch[0:64, 0:128], rhs=scratch[0:64, :],
            start=True, stop=True,
        )
    # The short-N tail must cover the full data-arrival jitter window: even
    # a ~300ns PE gap before the first real matmul resets the HAM window and
    # costs ~2us of half-clock time. Excess short warmups are nearly free
    # (~60ns each, and the PE stays busy for the gate either way).
    for _ in range(20):
        nc.tensor.matmul(
            ps_warm[:, 0:64], lhsT=scratch[0:64, 0:128], rhs=scratch[0:64, 0:64],
            start=True, stop=True,
        )

    for pair in range(BPC // 2):
        b0 = 2 * pair
        # Both images of the pair side by side: [2, CIN, NPAD] -> [128, NPAD].
        x_t = xpool.tile([128, NPAD], BF16, tag="x")
        xsrc = xp_ap[b0 : b0 + 2].rearrange("b c n -> (b c) n")
        if pair == 0:
            for s in range(len(xbounds) - 1):
                nc.sync.dma_start(
                    x_t[:, xbounds[s] : xbounds[s + 1]],
                    xsrc[:, xbounds[s] : xbounds[s + 1]],
                )
        else:
            # Pair 1's bulk load rides the ACT ring (idle after the weight
            # load) so the SP ring stays free for pair-0 slices and the
            # first output stores.
            nc.scalar.dma_start(x_t[:], xsrc[:])
        o_sb = [
            opool.tile([128, NOUT], BF16, tag="oA", name=f"oA_{pair}"),
            opool.tile([128, NOUT], BF16, tag="oB", name=f"oB_{pair}"),
        ]

        for g, grp in enumerate(groups):
            ps = {}
            for half in (0, 1):
                for c in grp:
                    ps[(half, c)] = psum.tile(
                        [128, CHUNK], F32, tag="ps", name=f"ps_{pair}_{half}_{c}"
                    )
            for t in range(TAPS):
                kh, kw = divmod(t, 3)
                o = kh * HP + kw
                st, sp = (t == 0), (t == TAPS - 1)
                for half in (0, 1):
                    lo, hi = 64 * half, 64 * half + 64
                    for c in grp:
                        nc.tensor.matmul(
                            ps[(half, c)][:],
                            lhsT=w_sb[lo:hi, t, :],
                            rhs=x_t[lo:hi, o + CHUNK * c : o + CHUNK * c + CHUNK],
                            start=st,
                            stop=sp,
                        )
            # Evict: compact 58-wide padded rows to 56-wide dense rows, add
            # the per-channel offset, and cast to bf16. Both images on DVE —
            # keeping ScalarE free of compute keeps the ACT DMA ring
            # unblocked (see the weight load above).
            # Image B first: its store rides the busier SP ring, so giving
            # its eviction the earlier DVE slot lets that store trigger
            # sooner (matters for the final chunk's drain).
            for c in grp:
                pb = ps[(1, c)].rearrange("p (r x) -> p r x", x=HP)[:, :, 0:HW]
                ob = o_sb[1][:, c * DCHUNK : (c + 1) * DCHUNK].rearrange(
                    "p (r x) -> p r x", x=HW
                )
                nc.vector.tensor_scalar_add(ob, pb, off_sb)
                pa = ps[(0, c)].rearrange("p (r x) -> p r x", x=HP)[:, :, 0:HW]
                oa = o_sb[0][:, c * DCHUNK : (c + 1) * DCHUNK].rearrange(
                    "p (r x) -> p r x", x=HW
                )
                nc.vector.tensor_scalar_add(oa, pa, off_sb)
            # Stream completed output slices out. Image A rides the ACT ring
            # (idle apart from evictions by the time stores begin), image B
            # the SP ring behind the input loads.
            if g in ogroup:
                si = ogroup.index(g)
                nc.scalar.dma_start(
                    out_ap[b0][:, obounds[si] : obounds[si + 1]],
                    o_sb[0][:, obounds[si] : obounds[si + 1]],
                )
                nc.sync.dma_start(
                    out_ap[b0 + 1][:, obounds[si] : obounds[si + 1]],
                    o_sb[1][:, obounds[si] : obounds[si + 1]],
                )


def _build_nc():
    global _NC_CACHE
    if _NC_CACHE is not None:
        return _NC_CACHE
    nc = bacc.Bacc(
        "TRN2", target_bir_lowering=False, debug=False, num_devices=NCORES
    )
    xp_ap = nc.dram_tensor("xp", [BPC, CIN, NPAD], BF16, kind="ExternalInput").ap()
    w2_ap = nc.dram_tensor("w2", [128, TAPS, K], BF16, kind="ExternalInput").ap()
    off_ap = nc.dram_tensor("off", [K, 1], F32, kind="ExternalInput").ap()
    out_ap = nc.dram_tensor("out", [BPC, K, NOUT], BF16, kind="ExternalOutput").ap()
    with tile.TileContext(nc) as tc:
        with ExitStack() as ctx:
            _conv_kernel(ctx, tc, out_ap, xp_ap, w2_ap, off_ap)
    nc.compile()
    _NC_CACHE = nc
    return nc


def _prep_inputs(x, weight, offset):
    """Host-side layout prep: pad x, transpose+duplicate weights, cast bf16."""
    x = np.asarray(x, dtype=np.float32)
    weight = np.asarray(weight, dtype=np.float32)
    offset = np.asarray(offset, dtype=np.float32)

    xph = np.zeros((B, CIN, NPAD), dtype=NPBF16)
    xph[:, :, : HP * HP].reshape(B, CIN, HP, HP)[:, :, 1 : 1 + HW, 1 : 1 + HW] = (
        x.astype(NPBF16)
    )

    wt = np.ascontiguousarray(weight.transpose(1, 2, 3, 0)).reshape(CIN, TAPS, K)
    w2 = np.concatenate([wt, wt], axis=0).astype(NPBF16)  # [128, 9, 128]
    off = np.ascontiguousarray(offset.reshape(K, 1))
    return xph, w2, off


def kernel(x, weight, offset):
    nc = _build_nc()
    xph, w2, off = _prep_inputs(x, weight, offset)
    in_maps = [
        {"xp": xph[i * BPC : (i + 1) * BPC], "w2": w2, "off": off}
        for i in range(NCORES)
    ]
    res = run_bass_kernel_spmd(nc, in_maps, list(range(NCORES))).results
    out = np.concatenate(
        [
            np.asarray(res[i]["out"]).reshape(BPC, K, HW, HW)
            for i in range(NCORES)
        ],
        axis=0,
    )
    return out.astype(np.float32)
